# revision 63
# baseline (speedup 1.0000x reference)
"""Bass/Tile TRN2 kernel for nn_AttnDecoder: attention decoder with LSTM cell.

Contract: kernel(**full_inputs) -> full output [B, S, OUT].
Shards batch B=128 over 8 NeuronCores (16 each), runs the sequential
scan fully on-device, gathers at the end.

v2 design (chain-latency + PE-instruction economy):
  - "^T" tensors put the feature dim on SBUF partitions, batch in free.
  - Big [*, S*BL] tensors use (s-major, b-minor) free order: col = s*16 + b.
  - Deferred softmax: scores -> exp (unnormalized) -> ctx~ accumulated via
    per-(b,chunk) matmuls with a ones-row giving z; 1/z folded into y_tilde
    via a tiny PE broadcast matmul. No transposes, no reduce chain.
  - Scores: per-chunk b-pair-packed stationaries [K=128(e), M=(2b,64s)],
    psum col = pair*2 + chunk, partitions = (b_local, s_local).
  - Gates: Whh head + Wih tail accumulate into ONE psum bank; activations
    read psum directly.
  - LSTM state convention: hT = 2h, cs = 2c (Wh/Wc/Whh and fc_out h-rows
    host-halved), which makes the cell 3 stt + 2 act + 1 copy.
  - fcy part of y_tilde hoisted: ytldY[64, S*BL] precomputed in init.
  - Optional DVE tanh-addition-formula path for the last FRML_S s-values
    (tanh(a+d) = (TE+th)/(1+TE*th), TE = tanh(encp) precomputed).
"""

import numpy as np
import ml_dtypes
import os as _os

B, S, E, D, OUT = 128, 128, 256, 256, 64
NCORES, BL = 8, 16
SB = S * BL
BF = ml_dtypes.bfloat16

NSTEPS = int(_os.environ.get("ATTN_NSTEPS", S))
# number of trailing s-positions computed via the DVE tanh-addition formula
FRML_S = int(_os.environ.get("ATTN_FRML_S", "0"))
# per (half, chunk) count of direct s-positions whose add runs on GPSIMD
POOL_S = int(_os.environ.get("ATTN_POOL_S", "0"))
# z-reduction on the GpSimd engine instead of a PE ones-matmul
POOLZ = int(_os.environ.get("ATTN_POOLZ", "1"))
NCH = 2            # s-chunks per step (64 s each)
SCH = S // NCH     # s per chunk

_built = None


def _host_prep(inputs):
    f32 = np.float32
    enc = np.ascontiguousarray(np.asarray(inputs["input_encoded"], f32))
    y = np.asarray(inputs["y_history"], f32)
    h0 = np.asarray(inputs["h0"], f32)
    c0 = np.asarray(inputs["c0"], f32)
    W1 = np.asarray(inputs["attn_W1"], f32)
    b1 = np.asarray(inputs["attn_b1"], f32)
    w2 = np.asarray(inputs["attn_w2"], f32)
    Wih = np.asarray(inputs["lstm_Wih"], f32)
    Whh = np.asarray(inputs["lstm_Whh"], f32)
    bg = np.asarray(inputs["lstm_bih"], f32) + np.asarray(inputs["lstm_bhh"], f32)
    fcW = np.asarray(inputs["fc_W"], f32)
    fcb = np.asarray(inputs["fc_b"], f32)
    foW = np.asarray(inputs["fc_out_W"], f32)

    Wh, Wc, We = W1[:D], W1[D : 2 * D], W1[2 * D :]
    # [i,f,g,o] -> [i,f,o,g]
    gp = np.concatenate([np.arange(0, 2 * D), np.arange(3 * D, 4 * D), np.arange(2 * D, 3 * D)])
    Wih_p, Whh_p, bg_p = Wih[:, gp].copy(), Whh[:, gp].copy(), bg[gp].copy()
    # double the g block so one tanh(0.5*x) activation serves all four gates
    Wih_p[:, 3 * D :] *= 2.0
    Whh_p[:, 3 * D :] *= 2.0
    bg_p[3 * D :] *= 2.0

    # state convention: hT = 2h, cs = 2c  ->  halve everything that
    # multiplies h or c.
    Whalf = 0.5 * Wh
    Chalf = 0.5 * Wc
    Whh_half = 0.5 * Whh_p
    foW_adj = foW.copy()
    foW_adj[:D] *= 0.5  # h-rows (ctx rows unchanged)

    shared = {
        "whc": np.stack(
            [Whalf[:128], Whalf[128:], Chalf[:128], Chalf[128:]]
        ).astype(BF),  # [4,128,E]
        "b1d": b1.reshape(2, 128, 1).astype(f32),
        "wed": np.stack([We[:128], We[128:]]).astype(BF),  # [2,128,E]
        "w2d": w2.reshape(2, 128, 1).astype(BF),
        "wihd": np.stack(
            [
                np.concatenate([Wih_p[:, m * 128 : (m + 1) * 128], bg_p[None, m * 128 : (m + 1) * 128]], 0)
                for m in range(8)
            ]
        ).astype(BF),  # [8,65,128]
        "whhd": Whh_half.reshape(2, 128, 8, 128).transpose(0, 2, 1, 3).copy().astype(BF),  # [2,8,128,128]
        "fcyd": np.concatenate([fcW[E:], fcb[None, :]], 0).astype(BF),  # [65,64]
        "fccd": np.stack([fcW[:128], fcW[128:256]]).astype(BF),  # [2,128,64]
        "woutd": foW_adj.reshape(4, 128, OUT * S).astype(BF),  # [4,128,8192]
        "id128d": np.eye(128, dtype=f32).astype(BF),
    }

    per_core = []
    for i in range(NCORES):
        sl = slice(i * BL, (i + 1) * BL)
        es = enc[sl]  # [16,S,E]
        ys = y[sl]  # [16,S,OUT]
        m = {
            "encTd": es.transpose(2, 1, 0).reshape(2, 128, SB).copy().astype(BF),
            "encNd": es.astype(BF),  # [16,128(s),256(e)]
            "yTd": np.concatenate(
                [ys.transpose(2, 1, 0).reshape(OUT, SB), np.ones((1, SB), f32)], 0
            ).astype(BF),  # [65, S*BL]
            "h0Td": (2.0 * h0[sl]).T.reshape(2, 128, BL).transpose(1, 0, 2).reshape(128, 32).copy().astype(BF),
            "c0Td": (2.0 * c0[sl]).T.reshape(2, 128, BL).transpose(1, 0, 2).reshape(128, 32).copy().astype(f32),
            "c0Tbd": (2.0 * c0[sl]).T.reshape(2, 128, BL).transpose(1, 0, 2).reshape(128, 32).copy().astype(BF),
        }
        m.update(shared)
        per_core.append(m)
    return per_core


def _build():
    global _built
    if _built is not None:
        return _built
    import concourse.bass as bass
    import concourse.mybir as mybir
    import concourse.tile as tile
    from concourse import bacc
    from contextlib import ExitStack

    dt = mybir.dt
    AF = mybir.ActivationFunctionType
    OP = mybir.AluOpType

    nc = bacc.Bacc("TRN2", target_bir_lowering=False, debug=False)

    d_encT = nc.dram_tensor("encTd", [2, 128, SB], dt.bfloat16, kind="ExternalInput")
    d_encN = nc.dram_tensor("encNd", [BL, 128, E], dt.bfloat16, kind="ExternalInput")
    d_yT = nc.dram_tensor("yTd", [65, SB], dt.bfloat16, kind="ExternalInput")
    d_h0T = nc.dram_tensor("h0Td", [128, 32], dt.bfloat16, kind="ExternalInput")
    d_c0T = nc.dram_tensor("c0Td", [128, 32], dt.float32, kind="ExternalInput")
    d_c0Tb = nc.dram_tensor("c0Tbd", [128, 32], dt.bfloat16, kind="ExternalInput")
    d_whc = nc.dram_tensor("whc", [4, 128, E], dt.bfloat16, kind="ExternalInput")
    d_b1 = nc.dram_tensor("b1d", [2, 128, 1], dt.float32, kind="ExternalInput")
    d_we = nc.dram_tensor("wed", [2, 128, E], dt.bfloat16, kind="ExternalInput")
    d_w2 = nc.dram_tensor("w2d", [2, 128, 1], dt.bfloat16, kind="ExternalInput")
    d_wih = nc.dram_tensor("wihd", [8, 65, 128], dt.bfloat16, kind="ExternalInput")
    d_whh = nc.dram_tensor("whhd", [2, 8, 128, 128], dt.bfloat16, kind="ExternalInput")
    d_fcy = nc.dram_tensor("fcyd", [65, OUT], dt.bfloat16, kind="ExternalInput")
    d_fcc = nc.dram_tensor("fccd", [2, 128, OUT], dt.bfloat16, kind="ExternalInput")
    d_wout = nc.dram_tensor("woutd", [4, 128, OUT * S], dt.bfloat16, kind="ExternalInput")
    d_id128 = nc.dram_tensor("id128d", [128, 128], dt.bfloat16, kind="ExternalInput")
    d_out = nc.dram_tensor("outd", [BL, OUT * S], dt.float32, kind="ExternalOutput")

    DBG_TS = [int(x) for x in _os.environ.get("ATTN_DBG_TS", "").split(",") if x]
    if DBG_TS:
        d_hs = nc.dram_tensor("dbg_hs", [len(DBG_TS), 128, 32], dt.bfloat16, kind="ExternalOutput")
        d_cs = nc.dram_tensor("dbg_cs", [len(DBG_TS), 128, 32], dt.float32, kind="ExternalOutput")
        d_ss = nc.dram_tensor("dbg_ss", [len(DBG_TS), 128, 16], dt.bfloat16, kind="ExternalOutput")
        d_yl = nc.dram_tensor("dbg_yl", [len(DBG_TS), 65, 16], dt.bfloat16, kind="ExternalOutput")
        d_gg = nc.dram_tensor("dbg_gg", [len(DBG_TS), 128, 128], dt.float32, kind="ExternalOutput")
        d_py = nc.dram_tensor("dbg_py", [len(DBG_TS), 65, 16], dt.float32, kind="ExternalOutput")

    with tile.TileContext(nc) as tc, ExitStack() as ctx:
        P = ctx.enter_context(tc.tile_pool(name="persist", bufs=1))

        def load(shape, dtype, src):
            t = P.tile(shape, dtype, tag=f"ld{load.n}", name=f"ld{load.n}")
            load.n += 1
            nc.sync.dma_start(t[:], src)
            return t

        load.n = 0

        encT = [load([128, SB], dt.bfloat16, d_encT[h]) for h in range(2)]
        encN = [load([128, E], dt.bfloat16, d_encN[b]) for b in range(BL)]
        yT = load([65, SB], dt.bfloat16, d_yT[:])
        whc = [load([128, E], dt.bfloat16, d_whc[i]) for i in range(4)]
        b1T = [load([128, 1], dt.float32, d_b1[h]) for h in range(2)]
        wesb = [load([128, E], dt.bfloat16, d_we[k]) for k in range(2)]
        w2sb = [load([128, 1], dt.bfloat16, d_w2[h]) for h in range(2)]
        wih = [load([65, 128], dt.bfloat16, d_wih[m]) for m in range(8)]
        whh = [[load([128, 128], dt.bfloat16, d_whh[k, m]) for m in range(8)] for k in range(2)]
        fcy = load([65, OUT], dt.bfloat16, d_fcy[:])
        fcc = [load([128, OUT], dt.bfloat16, d_fcc[h]) for h in range(2)]
        hT = load([128, 32], dt.bfloat16, d_h0T[:])
        cs = load([128, 32], dt.bfloat16, d_c0Tb[:])
        wout = [load([128, OUT * S], dt.bfloat16, d_wout[k]) for k in range(4)]

        encp = [P.tile([128, SB], dt.bfloat16, tag=f"encp{h}", name=f"encp{h}") for h in range(2)]
        TE = [P.tile([128, SB], dt.bfloat16, tag=f"TE{h}", name=f"TE{h}") for h in range(2)] if FRML_S else [None, None]
        encF = P.tile([128, BL * 65], dt.bfloat16, tag="encF", name="encF")
        ytldY = P.tile([OUT + 1, SB], dt.bfloat16, tag="ytldY", name="ytldY")
        nc.vector.memset(ytldY[64:65, :], 1.0)
        # GY[:, t*128 + m*16 + b] = (Wih^T [fc_y(y_t);1] + bias) for gate block m
        GY = P.tile([128, S * 128], dt.bfloat16, tag="GY", name="GY")
        ytld = P.tile([64, 16], dt.bfloat16, tag="ytld", name="ytld")
        id128 = load([128, 128], dt.bfloat16, d_id128[:])
        ones_row = P.tile([1, 128], dt.bfloat16, tag="ones_row", name="ones_row")
        nc.vector.memset(ones_row[:], 1.0)
        ones_col = P.tile([128, 1], dt.bfloat16, tag="ones_col", name="ones_col")
        nc.vector.memset(ones_col[:], 1.0)
        zrow = P.tile([1, 128], dt.bfloat16, tag="zrow", name="zrow")
        nc.vector.memset(zrow[:], 0.0)

        PS = ctx.enter_context(tc.tile_pool(name="psum", bufs=1, space="PSUM"))

        # ---- init: encp = We^T enc + b1; TE = tanh(encp); encF; ytldY ----
        for h in range(2):
            for nkc in range(4):
                ps = PS.tile([128, 512], dt.float32, tag="ep", name="ep", bufs=2)
                csl = slice(nkc * 512, (nkc + 1) * 512)
                for k in range(2):
                    nc.tensor.matmul(
                        ps[:],
                        wesb[k][:, h * 128 : (h + 1) * 128],
                        encT[k][:, csl],
                        start=(k == 0),
                        stop=(k == 1),
                    )
                nc.vector.tensor_scalar(encp[h][:, csl], ps[:], b1T[h][:], None, OP.add)
                if FRML_S:
                    nc.scalar.activation(TE[h][:, csl], encp[h][:, csl], AF.Tanh)

        # encF[:, b*65:(b+1)*65] = [enc_b @ fcc | ones]
        encF3 = encF[:].rearrange("p (b f) -> p b f", f=65)
        nc.vector.memset(encF3[:, :, 64], 1.0)
        encT3 = [encT[h][:].rearrange("p (s b) -> p s b", b=BL) for h in range(2)]
        for b in range(BL):
            pf = PS.tile([128, OUT], dt.float32, tag="yt", name="ef")
            for h in range(2):
                nc.tensor.matmul(
                    pf[:], encT3[h][:, :, b], fcc[h][:], start=(h == 0), stop=(h == 1)
                )
            nc.vector.tensor_copy(encF3[:, b, 0:64], pf[:])

        # ytldY = fcy^T yT (all steps at once)
        for nkc in range(4):
            csl = slice(nkc * 512, (nkc + 1) * 512)
            pyy = PS.tile([OUT, 512], dt.float32, tag="ep", name="yy", bufs=2)
            nc.tensor.matmul(pyy[:], fcy[:], yT[:, csl], start=True, stop=True)
            nc.vector.tensor_copy(ytldY[0:64, csl], pyy[:])

        # GY: y-history gate contributions (+lstm bias). Only the first
        # t-chunk (steps 0..31) is built upfront; the rest is drip-fed two
        # units per step into engine idle windows during steps 1..12.
        GY4 = GY[:].rearrange("p (t m b) -> p t m b", m=8, b=BL)

        def emit_gy_unit(m, nkc):
            csl = slice(nkc * 512, (nkc + 1) * 512)
            pgy = PS.tile([128, 512], dt.float32, tag="ep", name="gy", bufs=2)
            nc.tensor.matmul(pgy[:], wih[m][:], ytldY[:, csl], start=True, stop=True)
            nc.vector.tensor_copy(
                GY4[:, nkc * 32 : (nkc + 1) * 32, m, :],
                pgy[:].rearrange("p (t b) -> p t b", b=BL),
            )

        for m in range(8):
            emit_gy_unit(m, 0)
        gy_units = [(m, nkc) for nkc in range(1, 4) for m in range(8)]

        sp = ctx.enter_context(tc.tile_pool(name="step", bufs=2))

        # warm DVE/PE clocks on the state DMA deps
        nc.vector.tensor_copy(cb[:], cs[:])
        wrm = PS.tile([1, 16], dt.float32, tag="sa", name="wrm")
        nc.tensor.matmul(wrm[:], w2sb[0][:], hT[:, 0:16], start=True, stop=True)

        SD = S - FRML_S  # first SD s-values direct, rest formula

        for t in range(NSTEPS):
            # hc^T = Wh^T h + Wc^T c (+b1 already in encp) -> psum [128, 32]
            phc = PS.tile([128, 32], dt.float32, tag="hc", name="hc", bufs=2)
            hcb = sp.tile([128, 32], dt.bfloat16, tag="hcb", name="hcb")
            # zero-open the bank so both eh regions are continuations; all Wc
            # matmuls (gated only by cs) then run before the h2-gated Wh ones
            nc.tensor.matmul(phc[:], ones_row[:], zrow[:, 0:32], start=True, stop=False,
                             skip_group_check=True)
            for eh in range(2):
                o = phc[:, eh * 16 : (eh + 1) * 16]
                esl = slice(eh * 128, (eh + 1) * 128)
                nc.tensor.matmul(o, whc[2][:, esl], cs[:, 0:16], start=False, stop=False,
                                 skip_group_check=True)
                nc.tensor.matmul(o, whc[3][:, esl], cs[:, 16:32], start=False, stop=False,
                                 skip_group_check=True)
            for eh in range(2):
                o = phc[:, eh * 16 : (eh + 1) * 16]
                esl = slice(eh * 128, (eh + 1) * 128)
                nc.tensor.matmul(o, whc[0][:, esl], hT[:, 0:16], start=False, stop=False,
                                 skip_group_check=True)
                nc.tensor.matmul(o, whc[1][:, esl], hT[:, 16:32], start=False, stop=True,
                                 skip_group_check=True)
                nc.vector.tensor_copy(hcb[:, eh * 16 : (eh + 1) * 16], o)
            if FRML_S:
                th = sp.tile([128, 32], dt.bfloat16, tag="th", name="th")
                nc.scalar.activation(th[:], phc[:], AF.Tanh)

            # gates psum opens with the precomputed y-history part (+bias)
            pg = PS.tile([128, 128], dt.float32, tag="g", name="g")
            nc.tensor.matmul(pg[:], id128[:], GY[:, t * 128 : (t + 1) * 128],
                             start=True, stop=False, skip_group_check=True)
            for m in range(8):
                o = pg[:, m * 16 : (m + 1) * 16]
                nc.tensor.matmul(o, whh[0][m][:], hT[:, 0:16], start=False, stop=False,
                                 skip_group_check=True)
                nc.tensor.matmul(o, whh[1][m][:], hT[:, 16:32], start=False, stop=False,
                                 skip_group_check=True)

            # attention: chunked add/tanh -> scores -> exp -> ctx-accumulate
            Tt = [sp.tile([128, SB], dt.bfloat16, tag=f"T{h}", name=f"T{h}") for h in range(2)]
            pre = [sp.tile([128, SB], dt.bfloat16, tag=f"pre{h}", name=f"pre{h}") for h in range(2)]
            pscT = PS.tile([128, 16], dt.float32, tag="sa", name="scT")
            nc.tensor.matmul(pscT[:], ones_row[:], zrow[:, 0:16], start=True, stop=False,
                             skip_group_check=True)
            pT = sp.tile([128, 16], dt.bfloat16, tag="pT", name="pT")
            pyt = PS.tile([65, 16], dt.float32, tag="yt", name="yt")

            for h in range(2):
                for c in range(NCH):
                    s_lo, s_hi = c * SCH, (c + 1) * SCH
                    d_hi = min(SD, s_hi)  # direct sub-range end
                    if d_hi > s_lo:  # direct: pre = encp + hc; T = tanh(pre)
                        p_hi = min(s_lo + POOL_S, d_hi)  # Pool-assisted sub-range
                        if p_hi > s_lo:
                            csl = slice(s_lo * BL, p_hi * BL)
                            ns = p_hi - s_lo
                            hcv = hcb[:, None, h * 16 : (h + 1) * 16].to_broadcast((128, ns, BL))
                            pr3 = pre[h][:, csl].rearrange("p (s b) -> p s b", b=BL)
                            en3 = encp[h][:, csl].rearrange("p (s b) -> p s b", b=BL)
                            nc.gpsimd.tensor_tensor(pr3, en3, hcv, OP.add)
                        if d_hi > p_hi:
                            csl = slice(p_hi * BL, d_hi * BL)
                            ns = d_hi - p_hi
                            hcv = hcb[:, None, h * 16 : (h + 1) * 16].to_broadcast((128, ns, BL))
                            pr3 = pre[h][:, csl].rearrange("p (s b) -> p s b", b=BL)
                            en3 = encp[h][:, csl].rearrange("p (s b) -> p s b", b=BL)
                            nc.vector.tensor_tensor(pr3, en3, hcv, OP.add)
                        csl = slice(s_lo * BL, d_hi * BL)
                        nc.scalar.activation(Tt[h][:, csl], pre[h][:, csl], AF.Tanh)
                if h == 0 and FRML_S:
                    # th for the formula pass; placed here so it does not
                    # delay the first tanh on the ScalarE queue
                    th = sp.tile([128, 32], dt.bfloat16, tag="th", name="th")
                    nc.scalar.activation(th[:], phc[:], AF.Tanh)

            if FRML_S:
                # formula pass: T = (TE+th)/(1+TE*th), emitted AFTER all adds
                # so it never blocks them on the in-order DVE queue
                for h in range(2):
                    f_lo = SD
                    csl = slice(f_lo * BL, S * BL)
                    ns = S - f_lo
                    thv = th[:, None, h * 16 : (h + 1) * 16].to_broadcast((128, ns, BL))
                    te3 = TE[h][:, csl].rearrange("p (s b) -> p s b", b=BL)
                    pr3 = pre[h][:, csl].rearrange("p (s b) -> p s b", b=BL)
                    tt3 = Tt[h][:, csl].rearrange("p (s b) -> p s b", b=BL)
                    with nc.allow_low_precision(reason="bf16 attn"):
                        # Tt <- TE+th ; pre <- 1/(1+TE*th) ; Tt <- Tt*pre
                        nc.vector.tensor_tensor(tt3, te3, thv, OP.add)
                        nc.vector.tensor_tensor(pr3, te3, thv, OP.mult)
                        nc.vector.tensor_scalar(pre[h][:, csl], pre[h][:, csl], 1.0, None, OP.add)
                        nc.vector.reciprocal(pre[h][:, csl], pre[h][:, csl])
                        nc.vector.tensor_tensor(
                            Tt[h][:, csl], Tt[h][:, csl], pre[h][:, csl], OP.mult
                        )

            # scores: per-b full-s stationary [K=128(e), M=128(s)], halves
            # accumulate; h-outer emission so the h0 wave hides under h1 tanhs
            Ts = [Tt[h][:].rearrange("p (s b) -> p s b", b=BL) for h in range(2)]
            for h in range(2):
                for b in range(BL):
                    nc.tensor.matmul(
                        pscT[:, b : b + 1], Ts[h][:, :, b], w2sb[h][:],
                        start=False, stop=(h == 1), skip_group_check=True,
                    )

            # exp (psum -> sbuf bf16), unnormalized
            # single exp: the measured trace shows a column-split waits the
            # full score wave anyway (sem granularity) and only adds init cost
            nc.scalar.activation(pT[:], pscT[:], AF.Exp)

            # z off the PE queue: GpSimd partition ALL-reduce replicates the
            # sum on every partition, so one reciprocal over the full tile
            # yields the broadcast 1/z directly (DVE cost is free-size only)
            import concourse.bass_isa as bass_isa
            zred = sp.tile([128, 16], dt.float32, tag="zred", name="zred")
            nc.gpsimd.partition_all_reduce(
                zred[:], pT[:], channels=128, reduce_op=bass_isa.ReduceOp.add
            )
            rzs = sp.tile([128, 16], dt.bfloat16, tag="rzs", name="rzs")
            with nc.allow_low_precision(reason="softmax scale"):
                nc.vector.reciprocal(rzs[:], zred[:])

            # ctx~: per-b single matmul [K=128(s), M=65]
            for b in range(BL):
                nc.tensor.matmul(
                    pyt[:, b : b + 1],
                    encF3[:, b, :],
                    pT[:, b : b + 1],
                    start=True,
                    stop=True,
                    skip_group_check=True,
                )

            with nc.allow_low_precision(reason="bf16 ytld"):
                nc.vector.tensor_tensor(ytld[:], pyt[0:64, :], rzs[0:64, :], OP.mult)

            # gates tail: Wih on the ctx part only (y part + bias rode GY)
            for m in range(8):
                nc.tensor.matmul(
                    pg[:, m * 16 : (m + 1) * 16],
                    wih[m][0:64, :],
                    ytld[:],
                    start=False,
                    stop=True,
                    skip_group_check=True,
                )

            # LSTM cell on [128,(m,b)] psum. cols: i=[0:32] f=[32:64] o=[64:96]
            # g=[96:128] (g host-doubled, so one tanh(0.5x) serves all gates)
            thifo = sp.tile([128, 128], dt.bfloat16, tag="thifo", name="thifo")
            nc.scalar.activation(thifo[:], pg[:], AF.Tanh, scale=0.5)
            u = sp.tile([128, 32], dt.float32, tag="u", name="u")
            nc.vector.scalar_tensor_tensor(u[:], thifo[:, 32:64], 1.0, cs[:], OP.add, OP.mult)
            v = sp.tile([128, 32], dt.float32, tag="v", name="v")
            nc.vector.scalar_tensor_tensor(v[:], thifo[:, 0:32], 1.0, thifo[:, 96:128], OP.add, OP.mult)
            with nc.allow_low_precision(reason="bf16 c state"):
                nc.vector.scalar_tensor_tensor(cs[:], u[:], 0.5, v[:], OP.mult, OP.add)
            tcn = sp.tile([128, 32], dt.bfloat16, tag="tcn", name="tcn")
            nc.scalar.activation(tcn[:], cs[:], AF.Tanh, scale=0.5)
            with nc.allow_low_precision(reason="bf16 h"):
                nc.vector.scalar_tensor_tensor(hT[:], thifo[:, 64:96], 1.0, tcn[:], OP.add, OP.mult)
            nc.vector.tensor_copy(cb[:], cs[:])

            # drip-feed deferred GY init units into this step's idle windows
            for _ in range(2):
                if gy_units:
                    emit_gy_unit(*gy_units.pop(0))

            if t in DBG_TS:
                ix = DBG_TS.index(t)
                nc.sync.dma_start(d_hs[ix], hT[:])
                nc.sync.dma_start(d_cs[ix], cs[:])
                nc.sync.dma_start(d_ss[ix], pT[:])
                nc.sync.dma_start(d_yl[ix], ytld[:])
                ggs = sp.tile([128, 128], dt.float32, tag="dbg_g", name="dbg_g")
                nc.vector.tensor_copy(ggs[:], pg[:])
                nc.sync.dma_start(d_gg[ix], ggs[:])
                pys = sp.tile([65, 16], dt.float32, tag="dbg_p", name="dbg_p")
                nc.vector.tensor_copy(pys[:], pyt[:])
                nc.sync.dma_start(d_py[ix], pys[:])

            if t == NSTEPS - 1:
                # raw context (E=256), unnormalized then scaled by rz
                pcxT = PS.tile([128, 32], dt.float32, tag="cx", name="cxT")
                for b in range(BL):
                    for eh in range(2):
                        nc.tensor.matmul(
                            pcxT[:, eh * 16 + b : eh * 16 + b + 1],
                            encN[b][:, eh * 128 : (eh + 1) * 128],
                            pT[:, b : b + 1],
                            start=True,
                            stop=True,
                            skip_group_check=True,
                        )
                ctxT = P.tile([128, 32], dt.bfloat16, tag="ctxT", name="ctxT")
                with nc.allow_low_precision(reason="bf16 ctx"):
                    nc.vector.tensor_tensor(ctxT[:, 0:16], pcxT[:, 0:16], rzs[:], OP.mult)
                    nc.vector.tensor_tensor(ctxT[:, 16:32], pcxT[:, 16:32], rzs[:], OP.mult)

        # ---- final projection: out = [h|ctx] @ fc_out_W (fc_out_b on host) ----
        xch = [hT[:, 0:16], hT[:, 16:32], ctxT[:, 0:16], ctxT[:, 16:32]]
        for n in range(16):
            pf = PS.tile([16, 512], dt.float32, tag="ep", name="fin", bufs=2)
            csl = slice(n * 512, (n + 1) * 512)
            for k in range(4):
                nc.tensor.matmul(pf[:], xch[k], wout[k][:, csl], start=(k == 0), stop=(k == 3))
            ob = sp.tile([16, 512], dt.float32, tag="ob", name="ob", bufs=4)
            nc.vector.tensor_copy(ob[:], pf[:])
            nc.sync.dma_start(d_out[:, csl], ob[:])

    nc.compile()
    _built = nc
    return nc


def _install_ntff_hook():
    import sys
    import types

    if "antenv.axon_hooks" in sys.modules:
        return
    try:
        sys.path.insert(0, "/root/.axon_site/trn_agent_boot")
        from trn_boot import _ntff_profile_via_ctypes  # type: ignore

        hook = _ntff_profile_via_ctypes("/opt/axon/libaxon_pjrt.so")
    except Exception:
        hook = None
    mod = types.ModuleType("antenv.axon_hooks")
    mod._hook = hook
    mod.get_axon_ntff_profile_hook = lambda: mod._hook
    mod.set_axon_ntff_profile_hook = lambda h: setattr(mod, "_hook", h)
    sys.modules["antenv.axon_hooks"] = mod


def _run(inputs, trace=False, tmpdir=None):
    from concourse.bass_utils import run_bass_kernel_spmd

    if trace:
        _install_ntff_hook()

    nc = _build()
    in_maps = _host_prep(inputs)
    res = run_bass_kernel_spmd(
        nc, in_maps, list(range(NCORES)), trace=trace, tmpdir=tmpdir
    )
    out = np.concatenate([r["outd"] for r in res.results], axis=0)
    out = out + np.asarray(inputs["fc_out_b"], np.float32)[None, :]
    return out.reshape(B, S, OUT).astype(np.float32), res


def kernel(**inputs) -> np.ndarray:
    out, _ = _run(inputs, trace=False)
    return out
